# revision 1
# baseline (speedup 1.0000x reference)
"""AGCRNCell distributed Bass kernel for 8 TRN2 NeuronCores.

Sharding: graph-parallel over nodes (N=2048 -> 256 rows/core).
Each core computes its 256-row block of the support matrix (softmax rows),
diffuses with full X (k=1), and uses the Chebyshev identity
  S2 @ X = 2*sup@(sup@X) - X
with an AllGather of Y1 = sup@X to avoid materializing S2 = 2*sup@sup - I.
Per-node adaptive weights are applied factored through the embedding dim:
  out[b,n,o] = sum_d E[n,d] * (xg[b,n,:] @ wpool[d,:,:])[o] + (E@bpool)[n,o]
Matmuls run in bf16 (fp32 PSUM accumulate); softmax logits in fp32.
"""

import numpy as np
import ml_dtypes

import concourse.bass as bass
import concourse.mybir as mybir
import concourse.tile as tile
from concourse import bacc
from concourse.bass_utils import run_bass_kernel_spmd
from concourse.masks import make_identity

BF = mybir.dt.bfloat16
F32 = mybir.dt.float32

B, N, C, D, K = 16, 2048, 64, 10, 3
NCORES = 8
NB = N // NCORES     # 256 nodes per core
NT = NB // 128       # 2 nb-tiles per core
MT = N // 128        # 16 m-tiles
C2 = 2 * C           # 128
DO_G = D * C2        # 1280
DO_U = D * C         # 640

_CACHE = {}


def _build():
    core_ids = list(range(NCORES))
    nc = bacc.Bacc("TRN2", target_bir_lowering=False, debug=False,
                   num_devices=NCORES)

    def inp(name, shape, dt):
        return nc.dram_tensor(name, list(shape), dt, kind="ExternalInput").ap()

    xcat = inp("xcat", (B, N, C2), BF)        # concat(x, state) full
    xf = inp("xf", (B, N, C), BF)             # x full (for candidate)
    xcnb = inp("xcnb", (B, NB, C2), BF)       # this core's node rows of xcat
    xfnb = inp("xfnb", (B, NB, C), BF)        # this core's node rows of x
    etnb = inp("etnb", (D, NB), F32)          # E_nb^T (sharded)
    et = inp("et", (D, N), F32)               # E^T full
    enb = inp("enb", (NB, D), F32)            # E rows (sharded, d-contract)
    gw = inp("gw", (K * C2, DO_G), BF)        # gate wpool [kc, (d,o)]
    uw = inp("uw", (K * C2, DO_U), BF)
    gb = inp("gb", (NB, C2), F32)             # E@gate_bpool rows (sharded)
    ub = inp("ub", (NB, C), F32)
    lw = inp("lw", (C2, C), BF)               # lin_w.T
    lb = inp("lb", (128, C), F32)             # lin_b tiled
    out_ext = nc.dram_tensor("out", [B, NB, C], F32, kind="ExternalOutput").ap()

    AFT = mybir.ActivationFunctionType

    with tile.TileContext(nc) as tc:
        with (
            tc.tile_pool(name="const", bufs=1) as const,
            tc.tile_pool(name="sup", bufs=1) as supp,
            tc.tile_pool(name="xgTp", bufs=1) as xgtp,
            tc.tile_pool(name="work", bufs=2) as work,
            tc.tile_pool(name="xt", bufs=8) as xtp,
            tc.tile_pool(name="ev", bufs=4) as evp,
            tc.tile_pool(name="stat", bufs=4) as stat,
            tc.tile_pool(name="pDiff", bufs=4, space="PSUM") as pDiff,
            tc.tile_pool(name="pT", bufs=2, space="PSUM") as pT,
            tc.tile_pool(name="pW", bufs=2, space="PSUM") as pW,
            tc.tile_pool(name="dram", bufs=1, space="DRAM") as dram,
        ):
            ident = const.tile([128, 128], BF)
            make_identity(nc, ident[:])

            et_sb = const.tile([D, N], F32)
            nc.sync.dma_start(et_sb[:], et[:])
            etnb_sb = const.tile([D, NB], F32)
            nc.sync.dma_start(etnb_sb[:], etnb[:])
            enb_sb = [const.tile([128, D], F32, tag=f"enb{j}", name=f"enb_sb{j}") for j in range(NT)]
            gb_sb = [const.tile([128, C2], F32, tag=f"gb{j}", name=f"gb_sb{j}") for j in range(NT)]
            ub_sb = [const.tile([128, C], F32, tag=f"ub{j}", name=f"ub_sb{j}") for j in range(NT)]
            for j in range(NT):
                nc.sync.dma_start(enb_sb[j][:], enb[j * 128:(j + 1) * 128, :])
                nc.sync.dma_start(gb_sb[j][:], gb[j * 128:(j + 1) * 128, :])
                nc.sync.dma_start(ub_sb[j][:], ub[j * 128:(j + 1) * 128, :])
            gw_sb = [const.tile([128, DO_G], BF, tag=f"gw{k}", name=f"gw_sb{k}") for k in range(K)]
            uw_sb = [const.tile([128, DO_U], BF, tag=f"uw{k}", name=f"uw_sb{k}") for k in range(K)]
            for k in range(K):
                nc.sync.dma_start(gw_sb[k][:], gw[k * 128:(k + 1) * 128, :])
                nc.sync.dma_start(uw_sb[k][:], uw[k * 128:(k + 1) * 128, :])
            lw_sb = const.tile([C2, C], BF)
            nc.sync.dma_start(lw_sb[:], lw[:])
            lb_sb = const.tile([128, C], F32)
            nc.sync.dma_start(lb_sb[:], lb[:])

            # ---- support rows: sup[nb, m] = softmax(relu(E_nb @ E^T)) ----
            supT = [supp.tile([128, NB], BF, tag=f"supT{m}", name=f"supT{m}") for m in range(MT)]
            for j in range(NT):
                sraw = work.tile([128, N], F32, tag="sraw")
                for q in range(N // 512):
                    ps = pW.tile([128, 512], F32, tag="pw")
                    nc.tensor.matmul(
                        ps[:],
                        etnb_sb[:, j * 128:(j + 1) * 128],
                        et_sb[:, q * 512:(q + 1) * 512],
                    )
                    nc.scalar.activation(sraw[:, q * 512:(q + 1) * 512], ps[:],
                                         AFT.Relu)
                negmax = stat.tile([128, 1], F32, tag="negmax")
                nc.vector.tensor_reduce(negmax[:], sraw[:], mybir.AxisListType.X,
                                        mybir.AluOpType.max, negate=True)
                sexp = work.tile([128, N], F32, tag="sexp")
                rsum = stat.tile([128, 1], F32, tag="rsum")
                nc.scalar.activation(sexp[:], sraw[:], AFT.Exp,
                                     bias=negmax[:], accum_out=rsum[:])
                rinv = stat.tile([128, 1], F32, tag="rinv")
                nc.vector.reciprocal(rinv[:], rsum[:])
                sup_bf = work.tile([128, N], BF, tag="supbf")
                nc.scalar.activation(sup_bf[:], sexp[:], AFT.Copy, scale=rinv[:])
                for m in range(MT):
                    pt = pT.tile([128, 128], BF, tag="pt")
                    nc.tensor.transpose(pt[:], sup_bf[:, m * 128:(m + 1) * 128],
                                        ident[:])
                    nc.vector.tensor_copy(supT[m][:, j * 128:(j + 1) * 128],
                                          pt[:])

            # xgT_k: [128(c-of-k), B*NB (b-major)] bf16, persistent per call
            xgT = [xgtp.tile([128, B * NB], BF, tag=f"xgT{k}", name=f"xgT{k}") for k in range(K)]

            def diffusion(rhs_fill, dst_k, agin, sub_fill=None):
                """xg[nb, c] = sup_nb @ RHS for all b; rhs_fill(t, b0, m)
                fills t[128m, 512] with 4 batches b0..b0+3; result evicted,
                optionally DMA'd to agin and Chebyshev-combined via
                sub_fill(t, b0, j) (xg = 2*psum - sub). PE-transposed into
                xgT[dst_k]."""
                for half in range(2):          # 8 batches per half
                    for j in range(NT):
                        psd = [pDiff.tile([128, 512], F32, tag="pd", name=f"psd{g}")
                               for g in range(2)]
                        for m in range(MT):
                            for g in range(2):
                                t = xtp.tile([128, 512], BF, tag="rhs")
                                rhs_fill(t, half * 8 + g * 4, m)
                                nc.tensor.matmul(
                                    psd[g][:],
                                    supT[m][:, j * 128:(j + 1) * 128],
                                    t[:],
                                    start=(m == 0), stop=(m == MT - 1),
                                )
                        for g in range(2):
                            b0 = half * 8 + g * 4
                            xg = evp.tile([128, 512], BF, tag="xg")
                            if sub_fill is None:
                                nc.scalar.activation(xg[:], psd[g][:], AFT.Copy)
                            else:
                                sb2 = xtp.tile([128, 512], BF, tag="sub2")
                                sub_fill(sb2, b0, j)
                                t2 = evp.tile([128, 512], F32, tag="t2")
                                nc.scalar.activation(t2[:], psd[g][:], AFT.Copy,
                                                     scale=2.0)
                                nc.vector.tensor_sub(xg[:], t2[:], sb2[:])
                            for h in range(4):
                                b = b0 + h
                                sl = xg[:, h * 128:(h + 1) * 128]
                                if agin is not None:
                                    nc.sync.dma_start(
                                        agin[b, j * 128:(j + 1) * 128, :], sl)
                                pt = pT.tile([128, 128], BF, tag="pt")
                                nc.tensor.transpose(pt[:], sl, ident[:])
                                nc.vector.tensor_copy(
                                    xgT[dst_k][:, b * NB + j * 128:
                                               b * NB + j * 128 + 128],
                                    pt[:])

            def allgather(src, dst):
                nc.gpsimd.collective_compute(
                    "AllGather", mybir.AluOpType.bypass,
                    replica_groups=[core_ids],
                    ins=[src.opt()], outs=[dst.opt()],
                )

            zrin = dram.tile([B, NB, C], BF, tag="zrin")
            zrout = dram.tile([NCORES, B, NB, C], BF, tag="zrout")

            def weights_phase(is_gate):
                do = DO_G if is_gate else DO_U
                w_sb = gw_sb if is_gate else uw_sb
                o = C2 if is_gate else C
                bias_sb = gb_sb if is_gate else ub_sb
                for b in range(B):
                    for j in range(NT):
                        col = b * NB + j * 128
                        acc = evp.tile([128, o], F32, tag="acc")
                        nc.vector.tensor_copy(acc[:], bias_sb[j][:])
                        nq = (do + 511) // 512
                        for q in range(nq):
                            w = min(512, do - q * 512)
                            pw = pW.tile([128, w], F32, tag="pw")
                            for k in range(K):
                                nc.tensor.matmul(
                                    pw[:],
                                    xgT[k][:, col:col + 128],
                                    w_sb[k][:, q * 512:q * 512 + w],
                                    start=(k == 0), stop=(k == K - 1),
                                )
                            d0, d1 = (q * 512) // o, (q * 512 + w) // o
                            for d in range(d0, d1):
                                tmp = evp.tile([128, o], F32, tag="dtmp")
                                nc.scalar.activation(
                                    tmp[:], pw[:, d * o - q * 512:
                                               d * o - q * 512 + o],
                                    AFT.Copy, scale=enb_sb[j][:, d:d + 1])
                                nc.vector.tensor_add(acc[:], acc[:], tmp[:])
                        if is_gate:
                            sig = evp.tile([128, C2], BF, tag="sig")
                            nc.scalar.activation(sig[:], acc[:], AFT.Sigmoid)
                            pt = pT.tile([128, 128], BF, tag="pt")
                            nc.tensor.transpose(pt[:], sig[:], ident[:])
                            sigT = evp.tile([128, 128], BF, tag="sigT")
                            nc.vector.tensor_copy(sigT[:], pt[:])
                            pz = pT.tile([128, C], F32, tag="pt")
                            nc.tensor.matmul(pz[:], sigT[:], lw_sb[:])
                            zr = evp.tile([128, C], F32, tag="zrf")
                            nc.vector.tensor_add(zr[:], pz[:], lb_sb[:])
                            zrb = evp.tile([128, C], BF, tag="zrb")
                            nc.vector.tensor_copy(zrb[:], zr[:])
                            nc.sync.dma_start(
                                zrin[b, j * 128:(j + 1) * 128, :], zrb[:])
                        else:
                            fin = evp.tile([128, C], F32, tag="fin")
                            nc.scalar.activation(fin[:], acc[:], AFT.Tanh)
                            nc.sync.dma_start(
                                out_ext[b, j * 128:(j + 1) * 128, :], fin[:])

            # ================= call 1 (gate) =================
            def x1_fill(t, b0, m):
                for h in range(4):
                    nc.sync.dma_start(t[:, h * 128:(h + 1) * 128],
                                      xcat[b0 + h, m * 128:(m + 1) * 128, :])

            def nb1_fill(t, b0, j):
                for h in range(4):
                    nc.sync.dma_start(t[:, h * 128:(h + 1) * 128],
                                      xcnb[b0 + h, j * 128:(j + 1) * 128, :])

            # k=0: transpose node-block rows of xcat
            for b in range(B):
                for j in range(NT):
                    t0 = xtp.tile([128, C2], BF, tag="nbr")
                    nc.sync.dma_start(t0[:], xcnb[b, j * 128:(j + 1) * 128, :])
                    pt = pT.tile([128, 128], BF, tag="pt")
                    nc.tensor.transpose(pt[:], t0[:], ident[:])
                    nc.vector.tensor_copy(
                        xgT[0][:, b * NB + j * 128:b * NB + j * 128 + 128],
                        pt[:])
            agin1 = dram.tile([B, NB, C2], BF, tag="agin1")
            agout1 = dram.tile([NCORES, B, NB, C2], BF, tag="agout1")
            diffusion(x1_fill, 1, agin1)
            allgather(agin1, agout1)

            def y1_fill(t, b0, m):
                for h in range(4):
                    nc.sync.dma_start(
                        t[:, h * 128:(h + 1) * 128],
                        agout1[m // NT, b0 + h,
                               (m % NT) * 128:((m % NT) + 1) * 128, :])
            diffusion(y1_fill, 2, None, sub_fill=nb1_fill)
            weights_phase(True)
            allgather(zrin, zrout)

            # ================= call 2 (update) =================
            def x2_fill(t, b0, m):
                for h in range(4):
                    nc.sync.dma_start(t[:, h * 128:h * 128 + C],
                                      xf[b0 + h, m * 128:(m + 1) * 128, :])
                    nc.sync.dma_start(
                        t[:, h * 128 + C:(h + 1) * 128],
                        zrout[m // NT, b0 + h,
                              (m % NT) * 128:((m % NT) + 1) * 128, :])

            def nb2_fill(t, b0, j):
                for h in range(4):
                    nc.sync.dma_start(t[:, h * 128:h * 128 + C],
                                      xfnb[b0 + h, j * 128:(j + 1) * 128, :])
                    nc.sync.dma_start(t[:, h * 128 + C:(h + 1) * 128],
                                      zrin[b0 + h, j * 128:(j + 1) * 128, :])

            for b in range(B):
                for j in range(NT):
                    t0 = xtp.tile([128, C2], BF, tag="nbr")
                    nc.sync.dma_start(t0[:, 0:C],
                                      xfnb[b, j * 128:(j + 1) * 128, :])
                    nc.sync.dma_start(t0[:, C:C2],
                                      zrin[b, j * 128:(j + 1) * 128, :])
                    pt = pT.tile([128, 128], BF, tag="pt")
                    nc.tensor.transpose(pt[:], t0[:], ident[:])
                    nc.vector.tensor_copy(
                        xgT[0][:, b * NB + j * 128:b * NB + j * 128 + 128],
                        pt[:])
            agin2 = dram.tile([B, NB, C2], BF, tag="agin2")
            agout2 = dram.tile([NCORES, B, NB, C2], BF, tag="agout2")
            diffusion(x2_fill, 1, agin2)
            allgather(agin2, agout2)

            def y2_fill(t, b0, m):
                for h in range(4):
                    nc.sync.dma_start(
                        t[:, h * 128:(h + 1) * 128],
                        agout2[m // NT, b0 + h,
                               (m % NT) * 128:((m % NT) + 1) * 128, :])
            diffusion(y2_fill, 2, None, sub_fill=nb2_fill)
            weights_phase(False)

    nc.compile()
    return nc


def kernel(x, state, node_embeddings, gate_wpool, gate_bpool,
           upd_wpool, upd_bpool, lin_w, lin_b):
    x = np.asarray(x, np.float32)
    state = np.asarray(state, np.float32)
    E = np.asarray(node_embeddings, np.float32)
    bf = ml_dtypes.bfloat16

    if "nc" not in _CACHE:
        _CACHE["nc"] = _build()
    nc = _CACHE["nc"]

    xcat = np.concatenate([x, state], axis=-1).astype(bf)          # [B,N,128]
    xfb = x.astype(bf)
    et = np.ascontiguousarray(E.T)                                  # [10, N]
    gwr = np.asarray(gate_wpool, np.float32).transpose(1, 2, 0, 3) \
        .reshape(K * C2, DO_G).astype(bf)                           # [kc,(d,o)]
    uwr = np.asarray(upd_wpool, np.float32).transpose(1, 2, 0, 3) \
        .reshape(K * C2, DO_U).astype(bf)
    gbf = E @ np.asarray(gate_bpool, np.float32)                    # [N, 128]
    ubf = E @ np.asarray(upd_bpool, np.float32)                     # [N, 64]
    lwT = np.ascontiguousarray(np.asarray(lin_w, np.float32).T).astype(bf)
    lbT = np.tile(np.asarray(lin_b, np.float32)[None, :], (128, 1))

    in_maps = []
    for r in range(NCORES):
        sl = slice(r * NB, (r + 1) * NB)
        in_maps.append({
            "xcat": xcat, "xf": xfb,
            "xcnb": np.ascontiguousarray(xcat[:, sl, :]),
            "xfnb": np.ascontiguousarray(xfb[:, sl, :]),
            "etnb": np.ascontiguousarray(et[:, sl]),
            "et": et,
            "enb": np.ascontiguousarray(E[sl, :]),
            "gw": gwr, "uw": uwr,
            "gb": np.ascontiguousarray(gbf[sl, :]),
            "ub": np.ascontiguousarray(ubf[sl, :]),
            "lw": lwT, "lb": lbT,
        })
    global _LAST_IN_MAPS
    _LAST_IN_MAPS = in_maps
    res = run_bass_kernel_spmd(nc, in_maps, core_ids=list(range(NCORES)))
    outs = [res.results[r]["out"] for r in range(NCORES)]           # [B,NB,C]
    return np.concatenate(outs, axis=1).astype(np.float32)



# revision 13
# speedup vs baseline: 5.0273x; 5.0273x over previous
"""AGCRNCell distributed Bass kernel for 8 TRN2 NeuronCores.

Batch-parallel: B=16 -> 2 batches/core, zero collectives.  Each core:
  A = exp(relu(E @ E^T))      (symmetric -> A^T = A, no transposes;
                               softmax normalization deferred: S@v =
                               rinv * (A@v), rinv applied at eviction)
  diffusion hops as dense bf16 matmuls over 128x128 tiles of A,
  Chebyshev term folded into the weight pools host-side:
      sum_k xg_k w_k = xg0 (w0-w2) + y1 w1 + u2 (2 w2),
      y1 = rinv*(A@x), u2 = rinv*(A@y1)
  per-node adaptive weights factored through the embedding dim D=10:
      out[n,o] = sum_d E[n,d] * (xg[n,:] @ wpool[d,:,(o)])
  with weight pools laid out (o,d)-interleaved so the d-contraction is
  one DVE tensor_tensor(mult, E broadcast) + one tensor_reduce(X) per
  512-chunk.

All inputs are pre-tiled host-side into [128, W] row-major layouts so
every DMA moves multi-KB contiguous rows.
"""

import numpy as np
import ml_dtypes

import concourse.bass as bass
import concourse.mybir as mybir
import concourse.tile as tile
from concourse import bacc
from concourse.bass_utils import run_bass_kernel_spmd
from concourse.masks import make_identity

BF = mybir.dt.bfloat16
F32 = mybir.dt.float32
F32R = mybir.dt.float32r

B, N, C, D, K = 16, 2048, 64, 10, 3
NCORES = 8
B2 = B // NCORES          # 2 batches per core
MT = N // 128             # 16 row tiles
C2 = 2 * C                # 128
OG = 2 * C                # gate output width 128
OU = C                    # update output width 64
WOG = OG * D              # 1280 (o,d)-interleaved gate width
WOU = OU * D              # 640
GCH = [(0, 510), (510, 510), (1020, 260)]   # gate (o,d) chunks, mult of 10
UCH = [(0, 510), (510, 130)]                # update chunks

_CACHE = {}


def _build():
    nc = bacc.Bacc("TRN2", target_bir_lowering=False, debug=False,
                   num_devices=NCORES)

    def inp(name, shape, dt):
        return nc.dram_tensor(name, list(shape), dt, kind="ExternalInput").ap()

    xc_d = inp("xc", (128, MT * B2 * C2), BF)    # [p, (m,b,c2)] xcat tiles
    ew_d = inp("ew", (D, N), F32R)               # E^T
    et_d = inp("et", (128, MT * D), F32)         # [p, (j,d)] E rows
    gw_d = inp("gw", (128, K * WOG), BF)         # [c2, (k,o,d)] gate pool
    uw_d = inp("uw", (128, K * WOU), BF)
    gb_d = inp("gb", (128, MT * OG), BF)         # [p, (j,o)] E@gate_bpool
    ub_d = inp("ub", (128, MT * OU), BF)
    lw_d = inp("lw", (C2, C), BF)                # lin_w^T
    lb_d = inp("lb", (128, C), F32)              # lin_b tiled
    out_d = nc.dram_tensor("out", [128, B2 * MT * C], F32,
                           kind="ExternalOutput").ap()

    AFT = mybir.ActivationFunctionType
    MULT = mybir.AluOpType.mult
    ADD = mybir.AluOpType.add
    AXX = mybir.AxisListType.X

    with tile.TileContext(nc) as tc:
        with (
            tc.tile_pool(name="const", bufs=1) as const,
            tc.tile_pool(name="sraw", bufs=2) as srp,
            tc.tile_pool(name="stat", bufs=4) as stat,
            tc.tile_pool(name="xgp", bufs=3) as xgp,
            tc.tile_pool(name="scp", bufs=3) as scp,
            tc.tile_pool(name="accp", bufs=3) as accp,
            tc.tile_pool(name="sgp", bufs=3) as sgp,
            tc.tile_pool(name="pS", bufs=2, space="PSUM") as pS,
            tc.tile_pool(name="pD", bufs=2, space="PSUM") as pD,
            tc.tile_pool(name="pW", bufs=2, space="PSUM") as pW,
            tc.tile_pool(name="pT", bufs=2, space="PSUM") as pT,
        ):
            ident = const.tile([128, 128], BF)
            make_identity(nc, ident[:])

            EW = const.tile([D, N], F32R)
            nc.sync.dma_start(EW[:], ew_d[:])
            ET = const.tile([128, MT * D], F32)
            nc.sync.dma_start(ET[:], et_d[:])
            XC = const.tile([128, MT * B2 * C2], BF)
            nc.sync.dma_start(XC[:], xc_d[:])
            GB = const.tile([128, MT * OG], BF)
            nc.sync.dma_start(GB[:], gb_d[:])
            UB = const.tile([128, MT * OU], BF)
            nc.sync.dma_start(UB[:], ub_d[:])
            LW = const.tile([C2, C], BF)
            nc.sync.dma_start(LW[:], lw_d[:])
            LB = const.tile([128, C], F32)
            nc.sync.dma_start(LB[:], lb_d[:])
            GW = const.tile([128, K * WOG], BF)
            nc.sync.dma_start(GW[:], gw_d[:])
            UW = const.tile([128, K * WOU], BF)
            nc.sync.dma_start(UW[:], uw_d[:])

            A = [const.tile([128, N], BF, tag=f"A{j}", name=f"A{j}")
                 for j in range(MT)]
            RINV = const.tile([128, MT], F32)
            OUT = const.tile([128, B2 * MT * C], F32)

            # persistent diffusion state
            Y1 = const.tile([128, MT * B2 * C2], BF)    # rinv*(A@[x|s])
            U2 = const.tile([128, MT * B2 * C2], BF)    # rinv*(A@Y1)
            # stitched [x|zr], [y1x|y1z], [u2x|u2z] per (j, b): 64+64 cols
            XZ = const.tile([128, MT * B2 * C2], BF)
            YZ = const.tile([128, MT * B2 * C2], BF)
            UZ = const.tile([128, MT * B2 * C2], BF)

            def cat3(t, j):
                return t[:, j * 256:(j + 1) * 256] \
                    .rearrange("p (b c) -> p b c", b=B2)

            # ---- phase S: A = max(exp(E@E^T), 1), rinv = 1/rowsum ----
            # (exp(relu(x)) == max(exp(x), 1); clamp+rowsum fused on DVE)
            for j in range(MT):
                etmp = srp.tile([128, N], BF, tag="etmp")
                for q in range(N // 512):
                    ps = pS.tile([128, 512], F32, tag="ps")
                    nc.tensor.matmul(ps[:],
                                     EW[:, j * 128:(j + 1) * 128],
                                     EW[:, q * 512:(q + 1) * 512],
                                     start=True, stop=True)
                    nc.scalar.activation(etmp[:, q * 512:(q + 1) * 512],
                                         ps[:], AFT.Exp)
                zs = stat.tile([128, 1], F32, tag="zs")
                nc.vector.tensor_scalar(A[j][:], etmp[:], 1.0, 0.0,
                                        mybir.AluOpType.max,
                                        mybir.AluOpType.add,
                                        accum_out=zs[:])
                nc.vector.reciprocal(RINV[:, j:j + 1], zs[:])

            # ---- diffusion hop: dst_j = rinv_j * (A @ rhs) ----
            def hop_j(j, rhs_fn, evict_fn):
                pd = pD.tile([128, B2 * C2], F32, tag="pd")
                w = rhs_fn(0).free_size()
                for m in range(MT):
                    nc.tensor.matmul(pd[:, 0:w],
                                     A[m][:, j * 128:(j + 1) * 128],
                                     rhs_fn(m),
                                     start=(m == 0), stop=(m == MT - 1))
                evict_fn(pd[:, 0:w])

            def full_evict(dst, j):
                def ev(pdw):
                    nc.scalar.activation(dst[:, j * 256:(j + 1) * 256], pdw,
                                         AFT.Copy, scale=RINV[:, j:j + 1])
                return ev

            def z_evict(dst, j):
                # write z-halves into cols [64:128] of each 128-col group
                def ev(pdw):
                    nc.scalar.activation(
                        cat3(dst, j)[:, :, C:C2],
                        pdw.rearrange("p (b c) -> p b c", b=B2),
                        AFT.Copy, scale=RINV[:, j:j + 1])
                return ev

            for j in range(MT):
                hop_j(j, lambda m: XC[:, m * 256:(m + 1) * 256],
                      full_evict(Y1, j))
                # prefill x / y1x columns of the stitched tiles
                nc.scalar.activation(cat3(XZ, j)[:, :, 0:C],
                                     cat3(XC, j)[:, :, 0:C], AFT.Copy)
                nc.scalar.activation(cat3(YZ, j)[:, :, 0:C],
                                     cat3(Y1, j)[:, :, 0:C], AFT.Copy)

            # ---- weight application block ----
            def weight_block(is_gate, b, j):
                o = OG if is_gate else OU
                wsrc = GW if is_gate else UW
                wod = WOG if is_gate else WOU
                chunks = GCH if is_gate else UCH

                # lhsT = [xg0|xg1|xg2]^T via PE transposes into one PSUM
                # tile, single eviction
                srcs = (XC, Y1, U2) if is_gate else (XZ, YZ, UZ)
                pt = pT.tile([128, K * 128], BF, tag="pt")
                for k, src in enumerate(srcs):
                    nc.tensor.transpose(
                        pt[:, k * 128:(k + 1) * 128],
                        src[:, j * 256 + b * 128: j * 256 + (b + 1) * 128],
                        ident[:])
                xgT = xgp.tile([128, K * 128], BF, tag="xgT")
                nc.vector.tensor_copy(xgT[:], pt[:])

                acc = accp.tile([128, o], F32, tag=f"acc{o}")
                for (q0, w) in chunks:
                    pw = pW.tile([128, 510], F32, tag="pw")
                    for k in range(K):
                        nc.tensor.matmul(
                            pw[:, 0:w],
                            xgT[:, k * 128:(k + 1) * 128],
                            wsrc[:, k * wod + q0: k * wod + q0 + w],
                            start=(k == 0), stop=(k == K - 1))
                    sc = scp.tile([128, 510], BF, tag="sc")
                    e3 = ET[:, j * D:(j + 1) * D].unsqueeze(1) \
                        .broadcast_to([128, w // D, D])
                    nc.vector.tensor_tensor(
                        sc[:, 0:w].rearrange("p (o d) -> p o d", d=D),
                        pw[:, 0:w].rearrange("p (o d) -> p o d", d=D),
                        e3, MULT)
                    nc.vector.tensor_reduce(
                        acc[:, q0 // D: (q0 + w) // D],
                        sc[:, 0:w].rearrange("p (o d) -> p o d", d=D),
                        AXX, ADD)
                if is_gate:
                    nc.vector.tensor_add(acc[:], acc[:],
                                         GB[:, j * OG:(j + 1) * OG])
                    sig = sgp.tile([128, OG], BF, tag="sig")
                    nc.scalar.activation(sig[:], acc[:], AFT.Sigmoid)
                    pts = pT.tile([128, K * 128], BF, tag="pt")
                    nc.tensor.transpose(pts[:, 0:128], sig[:], ident[:])
                    sigT = sgp.tile([128, OG], BF, tag="sigT")
                    nc.vector.tensor_copy(sigT[:], pts[:, 0:128])
                    pz2 = pS.tile([128, 512], F32, tag="ps")
                    nc.tensor.matmul(pz2[:, 0:C], sigT[:], LW[:],
                                     start=True, stop=True)
                    # z_r written straight into [x|zr] cols [64:128]
                    nc.vector.tensor_add(
                        XZ[:, j * 256 + b * 128 + C: j * 256 + (b + 1) * 128],
                        pz2[:, 0:C], LB[:])
                else:
                    nc.vector.tensor_add(acc[:], acc[:],
                                         UB[:, j * OU:(j + 1) * OU])
                    nc.scalar.activation(
                        OUT[:, b * (MT * C) + j * C: b * (MT * C) + (j + 1) * C],
                        acc[:], AFT.Tanh)

            # D1b interleaved with the gate weight phase: PE streams U2
            # matmuls while DVE drains the previous block's d-contraction
            for j in range(MT):
                hop_j(j, lambda m: Y1[:, m * 256:(m + 1) * 256],
                      full_evict(U2, j))
                nc.scalar.activation(cat3(UZ, j)[:, :, 0:C],
                                     cat3(U2, j)[:, :, 0:C], AFT.Copy)
                for b in range(B2):
                    weight_block(True, b, j)

            def zr_rhs(m):
                return cat3(XZ, m)[:, :, C:C2]

            def y1z_rhs(m):
                return cat3(YZ, m)[:, :, C:C2]

            for j in range(MT):
                hop_j(j, zr_rhs, z_evict(YZ, j))
            for j in range(MT):
                hop_j(j, y1z_rhs, z_evict(UZ, j))
                for b in range(B2):
                    weight_block(False, b, j)

            nc.sync.dma_start(out_d[:], OUT[:])

    nc.compile()
    return nc


def kernel(x, state, node_embeddings, gate_wpool, gate_bpool,
           upd_wpool, upd_bpool, lin_w, lin_b):
    x = np.asarray(x, np.float32)
    state = np.asarray(state, np.float32)
    E = np.asarray(node_embeddings, np.float32)
    gw = np.asarray(gate_wpool, np.float32)
    gb = np.asarray(gate_bpool, np.float32)
    uw = np.asarray(upd_wpool, np.float32)
    ub = np.asarray(upd_bpool, np.float32)
    lw = np.asarray(lin_w, np.float32)
    lb = np.asarray(lin_b, np.float32)
    bf = ml_dtypes.bfloat16

    if "nc" not in _CACHE:
        _CACHE["nc"] = _build()
    nc = _CACHE["nc"]

    def fold_cheb(w):
        # w: [D, K, Ci, O] -> w0-w2, w1, 2*w2 then [c2, k, o, d] tiling
        wm = np.stack([w[:, 0] - w[:, 2], w[:, 1], 2.0 * w[:, 2]], axis=1)
        return np.ascontiguousarray(
            wm.transpose(2, 1, 3, 0).reshape(C2, -1)).astype(bf)

    gwr = fold_cheb(gw)                           # [128, 3840]
    uwr = fold_cheb(uw)                           # [128, 1920]
    gbf = (E @ gb).reshape(MT, 128, OG).transpose(1, 0, 2) \
        .reshape(128, MT * OG).astype(bf)
    ubf = (E @ ub).reshape(MT, 128, OU).transpose(1, 0, 2) \
        .reshape(128, MT * OU).astype(bf)
    etr = np.ascontiguousarray(E.T)               # [10, 2048] f32
    ett = E.reshape(MT, 128, D).transpose(1, 0, 2).reshape(128, MT * D)
    ett = np.ascontiguousarray(ett)
    lwT = np.ascontiguousarray(lw.T).astype(bf)   # [128, 64]
    lbt = np.ascontiguousarray(np.tile(lb[None, :], (128, 1)))

    xcat = np.concatenate([x, state], axis=-1)    # [16, 2048, 128] f32

    in_maps = []
    for r in range(NCORES):
        xcr = xcat[2 * r:2 * r + 2].reshape(B2, MT, 128, C2) \
            .transpose(2, 1, 0, 3).reshape(128, MT * B2 * C2)
        in_maps.append({
            "xc": np.ascontiguousarray(xcr).astype(bf),
            "ew": etr, "et": ett,
            "gw": gwr, "uw": uwr,
            "gb": gbf, "ub": ubf,
            "lw": lwT, "lb": lbt,
        })
    global _LAST_IN_MAPS
    _LAST_IN_MAPS = in_maps
    res = run_bass_kernel_spmd(nc, in_maps, core_ids=list(range(NCORES)))
    outs = []
    for r in range(NCORES):
        o = res.results[r]["out"]                  # [128, 2*16*64]
        o = o.reshape(128, B2, MT, C).transpose(1, 2, 0, 3) \
            .reshape(B2, N, C)
        outs.append(o)
    return np.concatenate(outs, axis=0).astype(np.float32)


# revision 20
# speedup vs baseline: 5.0543x; 1.0054x over previous
"""AGCRNCell distributed Bass kernel for 8 TRN2 NeuronCores.

Batch-parallel: B=16 -> 2 batches/core, zero collectives.  Each core:
  A = exp(relu(E @ E^T))      (symmetric -> A^T = A, no transposes;
                               softmax normalization deferred: S@v =
                               rinv * (A@v), rinv applied at eviction)
  diffusion hops as dense bf16 matmuls over 128x128 tiles of A,
  Chebyshev term folded into the weight pools host-side:
      sum_k xg_k w_k = xg0 (w0-w2) + y1 w1 + u2 (2 w2),
      y1 = rinv*(A@x), u2 = rinv*(A@y1)
  per-node adaptive weights factored through the embedding dim D=10:
      out[n,o] = sum_d E[n,d] * (xg[n,:] @ wpool[d,:,(o)])
  with weight pools laid out (o,d)-interleaved so the d-contraction is
  one DVE tensor_tensor(mult, E broadcast) + one tensor_reduce(X) per
  512-chunk.

All inputs are pre-tiled host-side into [128, W] row-major layouts so
every DMA moves multi-KB contiguous rows.
"""

import numpy as np
import ml_dtypes

import concourse.bass as bass
import concourse.mybir as mybir
import concourse.tile as tile
from concourse import bacc
from concourse.bass_utils import run_bass_kernel_spmd
from concourse.masks import make_identity

BF = mybir.dt.bfloat16
F32 = mybir.dt.float32
F32R = mybir.dt.float32r

B, N, C, D, K = 16, 2048, 64, 10, 3
NCORES = 8
B2 = B // NCORES          # 2 batches per core
MT = N // 128             # 16 row tiles
C2 = 2 * C                # 128
OG = 2 * C                # gate output width 128
OU = C                    # update output width 64
WOG = OG * D              # 1280 (o,d)-interleaved gate width
WOU = OU * D              # 640
GCH = [(0, 510), (510, 510), (1020, 260)]   # gate (o,d) chunks, mult of 10
UCH = [(0, 510), (510, 130)]                # update chunks

_CACHE = {}


def _build():
    nc = bacc.Bacc("TRN2", target_bir_lowering=False, debug=False,
                   num_devices=NCORES)

    def inp(name, shape, dt):
        return nc.dram_tensor(name, list(shape), dt, kind="ExternalInput").ap()

    xc_d = inp("xc", (128, MT * B2 * C2), BF)    # [p, (m,b,c2)] xcat tiles
    ew_d = inp("ew", (D, N), F32R)               # E^T
    et_d = inp("et", (128, MT * D), F32)         # [p, (j,d)] E rows
    gw_d = inp("gw", (128, K * WOG), BF)         # [c2, (k,o,d)] gate pool
    uw_d = inp("uw", (128, K * WOU), BF)
    gb_d = inp("gb", (128, MT * OG), BF)         # [p, (j,o)] E@gate_bpool
    ub_d = inp("ub", (128, MT * OU), BF)
    lw_d = inp("lw", (C2, C), BF)                # lin_w^T
    lb_d = inp("lb", (128, C), F32)              # lin_b tiled
    out_d = nc.dram_tensor("out", [128, B2 * MT * C], F32,
                           kind="ExternalOutput").ap()

    AFT = mybir.ActivationFunctionType
    MULT = mybir.AluOpType.mult
    ADD = mybir.AluOpType.add
    AXX = mybir.AxisListType.X

    with tile.TileContext(nc) as tc:
        with (
            tc.tile_pool(name="const", bufs=1) as const,
            tc.tile_pool(name="sraw", bufs=2) as srp,
            tc.tile_pool(name="stat", bufs=4) as stat,
            tc.tile_pool(name="xgp", bufs=3) as xgp,
            tc.tile_pool(name="scp", bufs=3) as scp,
            tc.tile_pool(name="accp", bufs=3) as accp,
            tc.tile_pool(name="sgp", bufs=3) as sgp,
            tc.tile_pool(name="pS", bufs=2, space="PSUM") as pS,
            tc.tile_pool(name="pD", bufs=2, space="PSUM") as pD,
            tc.tile_pool(name="pW", bufs=2, space="PSUM") as pW,
            tc.tile_pool(name="pT", bufs=2, space="PSUM") as pT,
        ):
            ident = const.tile([128, 128], BF)
            make_identity(nc, ident[:])

            EW = const.tile([D, N], F32R)
            nc.sync.dma_start(EW[:], ew_d[:])
            ET = const.tile([128, MT * D], F32)
            nc.sync.dma_start(ET[:], et_d[:])
            XC = const.tile([128, MT * B2 * C2], BF)
            nc.sync.dma_start(XC[:], xc_d[:])
            GB = const.tile([128, MT * OG], BF)
            nc.sync.dma_start(GB[:], gb_d[:])
            UB = const.tile([128, MT * OU], BF)
            nc.sync.dma_start(UB[:], ub_d[:])
            LW = const.tile([C2, C], BF)
            nc.sync.dma_start(LW[:], lw_d[:])
            LB = const.tile([128, C], F32)
            nc.sync.dma_start(LB[:], lb_d[:])
            GW = const.tile([128, K * WOG], BF)
            nc.sync.dma_start(GW[:], gw_d[:])
            UW = const.tile([128, K * WOU], BF)
            nc.sync.dma_start(UW[:], uw_d[:])

            A = [const.tile([128, N], BF, tag=f"A{j}", name=f"A{j}")
                 for j in range(MT)]
            RINV = const.tile([128, MT], F32)
            OUT = const.tile([128, B2 * MT * C], F32)

            # persistent diffusion state
            Y1 = const.tile([128, MT * B2 * C2], BF)    # rinv*(A@[x|s])
            U2 = const.tile([128, MT * B2 * C2], BF)    # rinv*(A@Y1)
            # stitched [x|zr], [y1x|y1z], [u2x|u2z] per (j, b): 64+64 cols
            XZ = const.tile([128, MT * B2 * C2], BF)
            YZ = const.tile([128, MT * B2 * C2], BF)
            UZ = const.tile([128, MT * B2 * C2], BF)

            def cat3(t, j):
                return t[:, j * 256:(j + 1) * 256] \
                    .rearrange("p (b c) -> p b c", b=B2)

            # ---- phase S: A = max(exp(E@E^T), 1), rinv = 1/rowsum ----
            # (exp(relu(x)) == max(exp(x), 1); clamp+rowsum fused on DVE)
            for j in range(MT):
                etmp = srp.tile([128, N], BF, tag="etmp")
                for q in range(N // 512):
                    ps = pS.tile([128, 512], F32, tag="ps")
                    nc.tensor.matmul(ps[:],
                                     EW[:, j * 128:(j + 1) * 128],
                                     EW[:, q * 512:(q + 1) * 512],
                                     start=True, stop=True)
                    nc.scalar.activation(etmp[:, q * 512:(q + 1) * 512],
                                         ps[:], AFT.Exp)
                zs = stat.tile([128, 1], F32, tag="zs")
                nc.vector.tensor_scalar(A[j][:], etmp[:], 1.0, 0.0,
                                        mybir.AluOpType.max,
                                        mybir.AluOpType.add,
                                        accum_out=zs[:])
                nc.vector.reciprocal(RINV[:, j:j + 1], zs[:])

            # ---- diffusion hop: dst_j = rinv_j * (A @ rhs) ----
            def hop_j(j, rhs_fn, evict_fn):
                pd = pD.tile([128, B2 * C2], F32, tag="pd")
                w = rhs_fn(0).free_size()
                for m in range(MT):
                    nc.tensor.matmul(pd[:, 0:w],
                                     A[m][:, j * 128:(j + 1) * 128],
                                     rhs_fn(m),
                                     start=(m == 0), stop=(m == MT - 1))
                evict_fn(pd[:, 0:w])

            def full_evict(dst, j):
                def ev(pdw):
                    nc.scalar.activation(dst[:, j * 256:(j + 1) * 256], pdw,
                                         AFT.Copy, scale=RINV[:, j:j + 1])
                return ev

            def z_evict(dst, j):
                # write z-halves into cols [64:128] of each 128-col group
                def ev(pdw):
                    nc.scalar.activation(
                        cat3(dst, j)[:, :, C:C2],
                        pdw.rearrange("p (b c) -> p b c", b=B2),
                        AFT.Copy, scale=RINV[:, j:j + 1])
                return ev

            for j in range(MT):
                hop_j(j, lambda m: XC[:, m * 256:(m + 1) * 256],
                      full_evict(Y1, j))
                # prefill x / y1x columns of the stitched tiles (gpsimd:
                # SBUF-to-SBUF, keeps scalar/DVE free)
                nc.gpsimd.tensor_copy(cat3(XZ, j)[:, :, 0:C],
                                      cat3(XC, j)[:, :, 0:C])
                nc.gpsimd.tensor_copy(cat3(YZ, j)[:, :, 0:C],
                                      cat3(Y1, j)[:, :, 0:C])

            # ---- weight application block ----
            def weight_block(is_gate, b, j):
                o = OG if is_gate else OU
                wsrc = GW if is_gate else UW
                wod = WOG if is_gate else WOU
                chunks = GCH if is_gate else UCH

                # lhsT = [xg0|xg1|xg2]^T via PE transposes into one PSUM
                # tile, single eviction
                srcs = (XC, Y1, U2) if is_gate else (XZ, YZ, UZ)
                pt = pT.tile([128, K * 128], BF, tag="pt")
                for k, src in enumerate(srcs):
                    nc.tensor.transpose(
                        pt[:, k * 128:(k + 1) * 128],
                        src[:, j * 256 + b * 128: j * 256 + (b + 1) * 128],
                        ident[:])
                xgT = xgp.tile([128, K * 128], BF, tag="xgT")
                nc.scalar.activation(xgT[:], pt[:], AFT.Copy)

                acc = accp.tile([128, o], F32, tag=f"acc{o}")
                for ci, (q0, w) in enumerate(chunks):
                    pw = pW.tile([128, 510], F32, tag="pw")
                    for k in range(K):
                        nc.tensor.matmul(
                            pw[:, 0:w],
                            xgT[:, k * 128:(k + 1) * 128],
                            wsrc[:, k * wod + q0: k * wod + q0 + w],
                            start=(k == 0), stop=(k == K - 1))
                    sc = scp.tile([128, 510], BF, tag="sc")
                    e3 = ET[:, j * D:(j + 1) * D].unsqueeze(1) \
                        .broadcast_to([128, w // D, D])
                    nc.vector.tensor_tensor(
                        sc[:, 0:w].rearrange("p (o d) -> p o d", d=D),
                        pw[:, 0:w].rearrange("p (o d) -> p o d", d=D),
                        e3, MULT)
                    nc.vector.tensor_reduce(
                        acc[:, q0 // D: (q0 + w) // D],
                        sc[:, 0:w].rearrange("p (o d) -> p o d", d=D),
                        AXX, ADD)
                if is_gate:
                    nc.vector.tensor_add(acc[:], acc[:],
                                         GB[:, j * OG:(j + 1) * OG])
                    sig = sgp.tile([128, OG], BF, tag="sig")
                    nc.scalar.activation(sig[:], acc[:], AFT.Sigmoid)
                    pts = pT.tile([128, K * 128], BF, tag="pt")
                    nc.tensor.transpose(pts[:, 0:128], sig[:], ident[:])
                    sigT = sgp.tile([128, OG], BF, tag="sigT")
                    nc.scalar.activation(sigT[:], pts[:, 0:128], AFT.Copy)
                    pz2 = pS.tile([128, 512], F32, tag="ps")
                    nc.tensor.matmul(pz2[:, 0:C], sigT[:], LW[:],
                                     start=True, stop=True)
                    # z_r written straight into [x|zr] cols [64:128]
                    nc.vector.tensor_add(
                        XZ[:, j * 256 + b * 128 + C: j * 256 + (b + 1) * 128],
                        pz2[:, 0:C], LB[:])
                else:
                    nc.vector.tensor_add(acc[:], acc[:],
                                         UB[:, j * OU:(j + 1) * OU])
                    nc.scalar.activation(
                        OUT[:, b * (MT * C) + j * C: b * (MT * C) + (j + 1) * C],
                        acc[:], AFT.Tanh)

            # D1b interleaved with the gate weight phase: PE streams U2
            # matmuls while DVE drains the previous block's d-contraction
            for j in range(MT):
                hop_j(j, lambda m: Y1[:, m * 256:(m + 1) * 256],
                      full_evict(U2, j))
                nc.gpsimd.tensor_copy(cat3(UZ, j)[:, :, 0:C],
                                      cat3(U2, j)[:, :, 0:C])
                for b in range(B2):
                    weight_block(True, b, j)

            def zr_rhs(m):
                return cat3(XZ, m)[:, :, C:C2]

            def y1z_rhs(m):
                return cat3(YZ, m)[:, :, C:C2]

            for j in range(MT):
                hop_j(j, zr_rhs, z_evict(YZ, j))
            for j in range(MT):
                hop_j(j, y1z_rhs, z_evict(UZ, j))
                for b in range(B2):
                    weight_block(False, b, j)

            nc.sync.dma_start(out_d[:], OUT[:])

    nc.compile()
    return nc


def kernel(x, state, node_embeddings, gate_wpool, gate_bpool,
           upd_wpool, upd_bpool, lin_w, lin_b):
    x = np.asarray(x, np.float32)
    state = np.asarray(state, np.float32)
    E = np.asarray(node_embeddings, np.float32)
    gw = np.asarray(gate_wpool, np.float32)
    gb = np.asarray(gate_bpool, np.float32)
    uw = np.asarray(upd_wpool, np.float32)
    ub = np.asarray(upd_bpool, np.float32)
    lw = np.asarray(lin_w, np.float32)
    lb = np.asarray(lin_b, np.float32)
    bf = ml_dtypes.bfloat16

    if "nc" not in _CACHE:
        _CACHE["nc"] = _build()
    nc = _CACHE["nc"]

    def fold_cheb(w):
        # w: [D, K, Ci, O] -> w0-w2, w1, 2*w2 then [c2, k, o, d] tiling
        wm = np.stack([w[:, 0] - w[:, 2], w[:, 1], 2.0 * w[:, 2]], axis=1)
        return np.ascontiguousarray(
            wm.transpose(2, 1, 3, 0).reshape(C2, -1)).astype(bf)

    gwr = fold_cheb(gw)                           # [128, 3840]
    uwr = fold_cheb(uw)                           # [128, 1920]
    gbf = (E @ gb).reshape(MT, 128, OG).transpose(1, 0, 2) \
        .reshape(128, MT * OG).astype(bf)
    ubf = (E @ ub).reshape(MT, 128, OU).transpose(1, 0, 2) \
        .reshape(128, MT * OU).astype(bf)
    etr = np.ascontiguousarray(E.T)               # [10, 2048] f32
    ett = E.reshape(MT, 128, D).transpose(1, 0, 2).reshape(128, MT * D)
    ett = np.ascontiguousarray(ett)
    lwT = np.ascontiguousarray(lw.T).astype(bf)   # [128, 64]
    lbt = np.ascontiguousarray(np.tile(lb[None, :], (128, 1)))

    xcat = np.concatenate([x, state], axis=-1)    # [16, 2048, 128] f32

    in_maps = []
    for r in range(NCORES):
        xcr = xcat[2 * r:2 * r + 2].reshape(B2, MT, 128, C2) \
            .transpose(2, 1, 0, 3).reshape(128, MT * B2 * C2)
        in_maps.append({
            "xc": np.ascontiguousarray(xcr).astype(bf),
            "ew": etr, "et": ett,
            "gw": gwr, "uw": uwr,
            "gb": gbf, "ub": ubf,
            "lw": lwT, "lb": lbt,
        })
    global _LAST_IN_MAPS
    _LAST_IN_MAPS = in_maps
    res = run_bass_kernel_spmd(nc, in_maps, core_ids=list(range(NCORES)))
    outs = []
    for r in range(NCORES):
        o = res.results[r]["out"]                  # [128, 2*16*64]
        o = o.reshape(128, B2, MT, C).transpose(1, 2, 0, 3) \
            .reshape(B2, N, C)
        outs.append(o)
    return np.concatenate(outs, axis=0).astype(np.float32)
